# revision 1
# baseline (speedup 1.0000x reference)
"""Self-contained Trainium2 Bass kernel: DeBERTa-style disentangled MHA.

Model (per reference):
    q = x @ Wq.T + bq ; k = x @ Wk.T + bk ; v = x @ Wv.T + bv   (per-head split)
    pos_k = rel_emb @ Wk.T + bk ; pos_q = rel_emb @ Wq.T + bq
    scores[i,j] = (q_i.k_j + A[i, i-j+s] + B[j, i-j+s]) * scale + mask
        where A[i,t] = q_i . pos_k[t],  B[j,t] = k_j . pos_q[t]
    out = softmax_j(scores) @ v

Sharding: 8-way head-parallel (2 heads/core), every core handles all 8 batch rows.
Scores are computed transposed (k index on partitions) so probs feed the PV matmul
directly; the softmax denominator comes from an appended ones-column on V.
The relative-position diagonal gathers ("shear") go through a DRAM round trip
in bf16: 384-wide windows are written with row pitch 512 and read back with
row pitch 511, which turns the per-row relative shift into a plain strided DMA.
"""

import numpy as np

B, S, DIM, H, HD = 8, 512, 1024, 16, 64
NCORES = 8
HPC = H // NCORES            # heads per core = 2
SCALE = float((HD * 3) ** -0.5)
W = 640                      # shear window width per 128-row tile
SEG = W * 128                # flat DRAM segment per tile

_prog_cache = {}


def _build_program():
    import concourse.bass as bass
    import concourse.mybir as mybir
    import concourse.tile as tile
    from concourse import bacc
    from concourse.masks import make_identity

    BF = mybir.dt.bfloat16
    F8 = mybir.dt.float8e4
    F32 = mybir.dt.float32
    AO = mybir.AluOpType
    AF = mybir.ActivationFunctionType

    nc = bacc.Bacc(None, target_bir_lowering=False, debug=False)

    def ap_of(t, extra_off, dims):
        return bass.AP(t.tensor, int(t.offset) + extra_off, dims)

    names = {}

    with tile.TileContext(nc) as tc:
        with tc.tile_pool(name="dram", bufs=1, space="DRAM") as dram, \
             tc.tile_pool(name="const", bufs=1) as const, \
             tc.tile_pool(name="persist", bufs=1) as persist:

            # ---------------- I/O ----------------
            xT_d = dram.tile([DIM, B * S], BF, kind="ExternalInput", name="xT")
            relT_d = dram.tile([DIM, 2 * S], BF, kind="ExternalInput", name="relT")
            wqT_d = dram.tile([DIM, 128], BF, kind="ExternalInput", name="wqT")
            wkT_d = dram.tile([DIM, 128], BF, kind="ExternalInput", name="wkT")
            wvT_d = dram.tile([DIM, 128], BF, kind="ExternalInput", name="wvT")
            bq_d = dram.tile([128], F32, kind="ExternalInput", name="bq")
            bk_d = dram.tile([128], F32, kind="ExternalInput", name="bk")
            bv_d = dram.tile([128], F32, kind="ExternalInput", name="bv")
            mask_d = dram.tile([B, S], F32, kind="ExternalInput", name="mask")
            out_d = dram.tile([B * HPC, HD + 1, S], F32, kind="ExternalOutput",
                              name="out")
            for k, t in [("xT", xT_d), ("relT", relT_d), ("wqT", wqT_d),
                         ("wkT", wkT_d), ("wvT", wvT_d), ("bq", bq_d),
                         ("bk", bk_d), ("bv", bv_d), ("mask", mask_d),
                         ("out", out_d)]:
                names[k] = t.name

            # ---------------- persistent SBUF ----------------
            ident = const.tile([128, 128], BF)
            make_identity(nc, ident)
            ident8 = const.tile([128, 128], F8)
            make_identity(nc, ident8)
            bq_sb = const.tile([128, 1], F32)
            bk_sb = const.tile([128, 1], F32)
            bv_sb = const.tile([128, 1], F32)
            nc.sync.dma_start(out=bq_sb, in_=bq_d.rearrange("(p o) -> p o", o=1))
            nc.sync.dma_start(out=bk_sb, in_=bk_d.rearrange("(p o) -> p o", o=1))
            nc.sync.dma_start(out=bv_sb, in_=bv_d.rearrange("(p o) -> p o", o=1))
            # mask_sb[p, b*4+J] = mask[b, 128J + p]
            mask_sb = const.tile([128, B, 4], F32)
            nc.sync.dma_start(
                out=mask_sb,
                in_=ap_of(mask_d, 0, [[1, 128], [S, B], [128, 4]]))

            QT = persist.tile([128, B * S], BF)       # (x@WqT + bq)*scale, transposed
            KT = persist.tile([128, B * S], BF)       # x@WkT + bk, transposed
            posKTr = persist.tile([128, 2 * S], BF)   # pos_k^T, t-axis reversed
            posQT = persist.tile([128, 2 * S], BF)    # (pos_q^T)*scale
            # Vaug[:, b*4+J, 65h : 65h+65] = [v rows | ones] for PV lhsT
            Vaug = persist.tile([128, B * 4, 130], BF)
            nc.vector.memset(Vaug[:, :, 64:65], 1.0)
            nc.vector.memset(Vaug[:, :, 129:130], 1.0)

            # ---------------- setup phase ----------------
            with tc.tile_pool(name="wpool", bufs=1) as wpool, \
                 tc.tile_pool(name="xpool", bufs=1) as xpool, \
                 tc.tile_pool(name="setup_sb", bufs=1) as ssb, \
                 tc.tile_pool(name="setup_ps", bufs=2, space="PSUM") as sps:

                wq_sb = wpool.tile([128, 8, 128], BF)
                wk_sb = wpool.tile([128, 8, 128], BF)
                wv_sb = wpool.tile([128, 8, 128], BF)
                for wsb, wd in [(wq_sb, wqT_d), (wk_sb, wkT_d), (wv_sb, wvT_d)]:
                    nc.sync.dma_start(
                        out=wsb, in_=wd.rearrange("(k p) o -> p k o", p=128))

                xch = []
                for k in range(8):
                    t = xpool.tile([128, B * S], BF, name=f"xch{k}", tag=f"xch{k}")
                    nc.sync.dma_start(out=t, in_=xT_d[128 * k:128 * k + 128, :])
                    xch.append(t)
                relch = []
                for k in range(8):
                    t = ssb.tile([128, 2 * S], BF, name=f"relch{k}", tag=f"relch{k}")
                    nc.sync.dma_start(out=t, in_=relT_d[128 * k:128 * k + 128, :])
                    relch.append(t)

                VT_sb = ssb.tile([128, B * S], BF)

                # Q/K/V projections, transposed outputs [o=128, s]
                for st in range(8):
                    sl = slice(512 * st, 512 * st + 512)
                    psq = sps.tile([128, 512], F32, tag="psq")
                    psk = sps.tile([128, 512], F32, tag="psk")
                    psv = sps.tile([128, 512], F32, tag="psv")
                    for k in range(8):
                        fl = dict(start=(k == 0), stop=(k == 7))
                        nc.tensor.matmul(psq, wq_sb[:, k, :], xch[k][:, sl], **fl)
                        nc.tensor.matmul(psk, wk_sb[:, k, :], xch[k][:, sl], **fl)
                        nc.tensor.matmul(psv, wv_sb[:, k, :], xch[k][:, sl], **fl)
                    nc.vector.tensor_scalar(QT[:, sl], psq, bq_sb, SCALE, AO.add, AO.mult)
                    nc.vector.tensor_scalar_add(KT[:, sl], psk, bk_sb)
                    nc.vector.tensor_scalar_add(VT_sb[:, sl], psv, bv_sb)

                # pos projections [o=128, t=1024]
                posKT_tmp = ssb.tile([128, 2 * S], BF)
                for tt in range(2):
                    sl = slice(512 * tt, 512 * tt + 512)
                    pspk = sps.tile([128, 512], F32, tag="psq")
                    pspq = sps.tile([128, 512], F32, tag="psk")
                    for k in range(8):
                        fl = dict(start=(k == 0), stop=(k == 7))
                        nc.tensor.matmul(pspk, wk_sb[:, k, :], relch[k][:, sl], **fl)
                        nc.tensor.matmul(pspq, wq_sb[:, k, :], relch[k][:, sl], **fl)
                    nc.vector.tensor_scalar_add(posKT_tmp[:, sl], pspk, bk_sb)
                    nc.vector.tensor_scalar(posQT[:, sl], pspq, bq_sb, SCALE,
                                            AO.add, AO.mult)
                # reversed copy: posKTr[:, t] = posKT_tmp[:, 1023 - t]
                nc.vector.tensor_copy(
                    posKTr,
                    ap_of(posKT_tmp, 2 * S - 1, [[2 * S, 128], [-1, 2 * S]]))

                # V transposes -> Vaug
                for b in range(B):
                    for J in range(4):
                        pvt = sps.tile([128, 128], F32, tag="psv")
                        c0 = 512 * b + 128 * J
                        nc.tensor.matmul(pvt, VT_sb[:, c0:c0 + 128], ident,
                                         start=True, stop=True)
                        nc.vector.tensor_copy(Vaug[:, 4 * b + J, 0:64], pvt[:, 0:64])
                        nc.vector.tensor_copy(Vaug[:, 4 * b + J, 65:129], pvt[:, 64:128])

            # ---------------- attention phase ----------------
            with tc.tile_pool(name="work", bufs=1) as work, \
                 tc.tile_pool(name="dscratch", bufs=1, space="DRAM") as dscratch, \
                 tc.tile_pool(name="psab", bufs=2, space="PSUM") as psab, \
                 tc.tile_pool(name="psqk", bufs=2, space="PSUM") as psqk, \
                 tc.tile_pool(name="pspv", bufs=2, space="PSUM") as pspv:

                WH = 384                   # half-window width (valid data)
                WP = 512                   # padded row pitch (512B lines, fp8)
                SEG2 = WP * 128            # flat segment per (I, jhalf) block
                for b in range(B):
                    # ABsb[h][:, seg, :] : segs 0-7 A windows, 8-15 B windows
                    ABsb, abflat, gath = {}, {}, {}
                    for h in range(HPC):
                        ABsb[h] = work.tile([128, 16, WP], BF, name=f"ABsb{h}",
                                            tag=f"ABsb{h}", bufs=3)
                        abflat[h] = dscratch.tile([16 * SEG2], BF,
                                                  name=f"abflat{h}",
                                                  tag=f"abflat{h}", bufs=3)
                        # gath[h][:, 0] = c2p (natural), [:, 1] = p2c^T
                        gath[h] = work.tile([128, 2, 4, 512], BF, name=f"gath{h}",
                                            tag=f"gath{h}", bufs=3)

                    # A = q . pos_k_rev windows ; B = k . pos_q windows
                    # 384-wide half-windows -> each psum tile is one bank
                    for m in range(2):
                        lhs = QT if m == 0 else KT
                        rhs = posKTr if m == 0 else posQT
                        for I in range(4):
                            for jh in range(2):
                                w0 = 384 - 128 * I + 256 * jh
                                ps = {}
                                for h in range(HPC):
                                    hp = slice(64 * h, 64 * h + 64)
                                    ps[h] = psab.tile(
                                        [128, WH], F32, name=f"psAB{h}",
                                        tag="psAB", bufs=3)
                                    lw = lhs[hp, 512 * b + 128 * I:
                                             512 * b + 128 * I + 128]
                                    nc.tensor.matmul(ps[h], lw,
                                                     rhs[hp, w0:w0 + WH],
                                                     start=True, stop=True,
                                                     tile_position=(64 * h, 0))
                                seg = m * 8 + I * 2 + jh
                                nc.vector.tensor_copy(ABsb[0][:, seg, 0:WH], ps[0])
                                nc.scalar.copy(ABsb[1][:, seg, 0:WH], ps[1])

                    # shear round trip: one fully-contiguous write + two
                    # strided gather-reads per head.  Writes go through SWDGE
                    # (gpsimd) to keep the SP sequencer free for the reads.
                    for h in range(HPC):
                        nc.gpsimd.dma_start(
                            out=ap_of(abflat[h], 0,
                                      [[WP, 128], [SEG2, 16], [1, WP]]),
                            in_=ABsb[h][:])
                        nc.sync.dma_start(
                            out=gath[h][:, 0],
                            in_=ap_of(abflat[h], 127,
                                      [[WP - 1, 128], [SEG2, 8], [1, 256]]))
                        nc.sync.dma_start(
                            out=gath[h][:, 1],
                            in_=ap_of(abflat[h], 8 * SEG2 + 128,
                                      [[WP - 1, 128], [SEG2, 8], [1, 256]]))

                    # scores (transposed), softmax, PV; heads interleaved so
                    # the K=64 qk matmuls pack into disjoint row groups
                    pvps = {}
                    for h in range(HPC):
                        pvps[h] = pspv.tile([65, 512], F32, name=f"pv{h}",
                                            tag=f"pv{h}", bufs=1)
                    for J in range(4):
                        qkps = {}
                        for h in range(HPC):
                            hp = slice(64 * h, 64 * h + 64)
                            qkps[h] = psqk.tile([128, 512], F32, name=f"qk{h}",
                                                tag="qk", bufs=3)
                            nc.tensor.matmul(
                                qkps[h],
                                KT[hp, 512 * b + 128 * J: 512 * b + 128 * J + 128],
                                QT[hp, 512 * b: 512 * b + 512],
                                start=True, stop=False,
                                tile_position=(64 * h, 0))
                        for h in range(HPC):
                            for I in range(4):
                                nc.tensor.matmul(
                                    qkps[h][:, 128 * I:128 * I + 128],
                                    gath[h][:, 0, I, 128 * J:128 * J + 128],
                                    ident, start=False, stop=False,
                                    skip_group_check=True)
                            nc.tensor.matmul(qkps[h], ident, gath[h][:, 1, J, :],
                                             start=False, stop=True)
                        for h in range(HPC):
                            PT = work.tile([128, 512], BF, name=f"PT{h}",
                                           tag=f"PT{h}", bufs=2)
                            nc.scalar.activation(
                                PT, qkps[h], AF.Exp,
                                bias=mask_sb[:, b, J:J + 1], scale=1.0)
                            nc.tensor.matmul(pvps[h],
                                             Vaug[:, 4 * b + J, 65 * h:65 * h + 65],
                                             PT, start=(J == 0), stop=(J == 3))
                    for h in range(HPC):
                        outsb = work.tile([65, 512], F32, name=f"outsb{h}",
                                          tag=f"outsb{h}", bufs=2)
                        nc.vector.tensor_copy(outsb, pvps[h])
                        nc.scalar.dma_start(out=out_d[HPC * b + h], in_=outsb)

    nc.compile()
    return nc, names


def _get_program():
    if "prog" not in _prog_cache:
        _prog_cache["prog"] = _build_program()
    return _prog_cache["prog"]


def _host_prep(x, rel_embeddings, attn_mask, Wq, bq, Wk, bk, Wv, bv):
    import ml_dtypes
    bf = ml_dtypes.bfloat16
    x = np.asarray(x, np.float32)
    xT = np.ascontiguousarray(x.reshape(B * S, DIM).T).astype(bf)
    relT = np.ascontiguousarray(np.asarray(rel_embeddings, np.float32).T).astype(bf)
    WqT = np.asarray(Wq, np.float32).T
    WkT = np.asarray(Wk, np.float32).T
    WvT = np.asarray(Wv, np.float32).T
    mask = np.ascontiguousarray(
        np.asarray(attn_mask, np.float32).reshape(B, S))
    bq = np.asarray(bq, np.float32)
    bk = np.asarray(bk, np.float32)
    bv = np.asarray(bv, np.float32)
    maps = []
    for c in range(NCORES):
        sl = slice(128 * c, 128 * c + 128)
        maps.append({
            "xT": xT,
            "relT": relT,
            "wqT": np.ascontiguousarray(WqT[:, sl]).astype(bf),
            "wkT": np.ascontiguousarray(WkT[:, sl]).astype(bf),
            "wvT": np.ascontiguousarray(WvT[:, sl]).astype(bf),
            "bq": np.ascontiguousarray(bq[sl]),
            "bk": np.ascontiguousarray(bk[sl]),
            "bv": np.ascontiguousarray(bv[sl]),
            "mask": mask,
        })
    return maps


def kernel(x, rel_embeddings, attn_mask, Wq, bq, Wk, bk, Wv, bv):
    from concourse.bass_utils import run_bass_kernel_spmd

    nc, names = _get_program()
    maps = _host_prep(x, rel_embeddings, attn_mask, Wq, bq, Wk, bk, Wv, bv)
    in_maps = [{names[k]: v for k, v in m.items()} for m in maps]
    res = run_bass_kernel_spmd(nc, in_maps, list(range(NCORES)))
    out = np.empty((B, S, DIM), np.float32)
    for c in range(NCORES):
        o = np.asarray(res.results[c][names["out"]], np.float32)
        for b in range(B):
            for hl in range(HPC):
                d0 = 128 * c + 64 * hl
                blk = o[HPC * b + hl]          # [65, 512]: rows 0-63 PV, row 64 L
                out[b, :, d0:d0 + 64] = (blk[0:64] / blk[64:65]).T
    return out



# revision 2
# speedup vs baseline: 1.1819x; 1.1819x over previous
"""Self-contained Trainium2 Bass kernel: DeBERTa-style disentangled MHA.

Model (per reference):
    q = x @ Wq.T + bq ; k = x @ Wk.T + bk ; v = x @ Wv.T + bv   (per-head split)
    pos_k = rel_emb @ Wk.T + bk ; pos_q = rel_emb @ Wq.T + bq
    scores[i,j] = (q_i.k_j + A[i, i-j+s] + B[j, i-j+s]) * scale + mask
        where A[i,t] = q_i . pos_k[t],  B[j,t] = k_j . pos_q[t]
    out = softmax_j(scores) @ v

Sharding: 8-way head-parallel (2 heads/core), every core handles all 8 batch rows.
Scores are computed transposed (k index on partitions) so probs feed the PV matmul
directly; the softmax denominator comes from an appended ones-column on V.
The relative-position diagonal gathers ("shear") go through a DRAM round trip
in fp8e4m3: 640-wide windows are written with row pitch 640 and read back with
row pitch 639, which turns the per-row relative shift into a plain strided DMA.
The batch loop is software-pipelined two deep (A/B windows for batch b+2 are
issued before the qk/softmax phase of batch b) so the PE never sits idle
waiting on the shear round trip.
"""

import numpy as np

B, S, DIM, H, HD = 8, 512, 1024, 16, 64
NCORES = 8
HPC = H // NCORES            # heads per core = 2
SCALE = float((HD * 3) ** -0.5)
W = 640                      # shear window width per 128-row tile
SEG = W * 128                # flat DRAM segment per (m, I) block, fp8 bytes
SKEW = 2                     # batches of A/B-window lead over the qk phase

_prog_cache = {}


def _build_program():
    import concourse.bass as bass
    import concourse.mybir as mybir
    import concourse.tile as tile
    from concourse import bacc
    from concourse.masks import make_identity

    BF = mybir.dt.bfloat16
    F8 = mybir.dt.float8e4
    F32 = mybir.dt.float32
    AO = mybir.AluOpType
    AF = mybir.ActivationFunctionType

    nc = bacc.Bacc(None, target_bir_lowering=False, debug=False)

    def ap_of(t, extra_off, dims):
        return bass.AP(t.tensor, int(t.offset) + extra_off, dims)

    names = {}

    with tile.TileContext(nc) as tc:
        with tc.tile_pool(name="dram", bufs=1, space="DRAM") as dram, \
             tc.tile_pool(name="const", bufs=1) as const, \
             tc.tile_pool(name="persist", bufs=1) as persist:

            # ---------------- I/O ----------------
            xT_d = dram.tile([DIM, B * S], BF, kind="ExternalInput", name="xT")
            relT_d = dram.tile([DIM, 2 * S], BF, kind="ExternalInput", name="relT")
            wqT_d = dram.tile([DIM, 128], BF, kind="ExternalInput", name="wqT")
            wkT_d = dram.tile([DIM, 128], BF, kind="ExternalInput", name="wkT")
            wvT_d = dram.tile([DIM, 128], BF, kind="ExternalInput", name="wvT")
            bq_d = dram.tile([128], F32, kind="ExternalInput", name="bq")
            bk_d = dram.tile([128], F32, kind="ExternalInput", name="bk")
            bv_d = dram.tile([128], F32, kind="ExternalInput", name="bv")
            mask_d = dram.tile([B, S], F32, kind="ExternalInput", name="mask")
            out_d = dram.tile([B * HPC, HD + 1, S], BF, kind="ExternalOutput",
                              name="out")
            for k, t in [("xT", xT_d), ("relT", relT_d), ("wqT", wqT_d),
                         ("wkT", wkT_d), ("wvT", wvT_d), ("bq", bq_d),
                         ("bk", bk_d), ("bv", bv_d), ("mask", mask_d),
                         ("out", out_d)]:
                names[k] = t.name

            # ---------------- persistent SBUF ----------------
            ident8 = const.tile([128, 128], F8)
            make_identity(nc, ident8)
            ident = const.tile([128, 128], BF)
            make_identity(nc, ident)
            bq_sb = const.tile([128, 1], F32)
            bk_sb = const.tile([128, 1], F32)
            bv_sb = const.tile([128, 1], F32)
            nc.sync.dma_start(out=bq_sb, in_=bq_d.rearrange("(p o) -> p o", o=1))
            nc.sync.dma_start(out=bk_sb, in_=bk_d.rearrange("(p o) -> p o", o=1))
            nc.sync.dma_start(out=bv_sb, in_=bv_d.rearrange("(p o) -> p o", o=1))
            # mask_sb[p, b*4+J] = mask[b, 128J + p]
            mask_sb = const.tile([128, B, 4], F32)
            nc.sync.dma_start(
                out=mask_sb,
                in_=ap_of(mask_d, 0, [[1, 128], [S, B], [128, 4]]))

            QT = persist.tile([128, B * S], BF)       # (x@WqT + bq)*scale, transposed
            KT = persist.tile([128, B * S], BF)       # x@WkT + bk, transposed
            posKTr = persist.tile([128, 2 * S], BF)   # pos_k^T, t-axis reversed
            posQT = persist.tile([128, 2 * S], BF)    # (pos_q^T)*scale
            # Vaug[:, b*4+J, 65h : 65h+65] = [v rows | ones] for PV lhsT
            Vaug = persist.tile([128, B * 4, 130], BF)
            nc.vector.memset(Vaug[:, :, 64:65], 1.0)
            nc.vector.memset(Vaug[:, :, 129:130], 1.0)

            # ---------------- setup phase ----------------
            with tc.tile_pool(name="wpool", bufs=1) as wpool, \
                 tc.tile_pool(name="xpool", bufs=1) as xpool, \
                 tc.tile_pool(name="setup_sb", bufs=1) as ssb, \
                 tc.tile_pool(name="setup_ps", bufs=2, space="PSUM") as sps:

                wq_sb = wpool.tile([128, 8, 128], BF)
                wk_sb = wpool.tile([128, 8, 128], BF)
                wv_sb = wpool.tile([128, 8, 128], BF)
                for wsb, wd in [(wq_sb, wqT_d), (wk_sb, wkT_d), (wv_sb, wvT_d)]:
                    nc.sync.dma_start(
                        out=wsb, in_=wd.rearrange("(k p) o -> p k o", p=128))

                xch = []
                for k in range(8):
                    t = xpool.tile([128, B * S], BF, name=f"xch{k}", tag=f"xch{k}")
                    nc.sync.dma_start(out=t, in_=xT_d[128 * k:128 * k + 128, :])
                    xch.append(t)
                relch = []
                for k in range(8):
                    t = ssb.tile([128, 2 * S], BF, name=f"relch{k}", tag=f"relch{k}")
                    nc.sync.dma_start(out=t, in_=relT_d[128 * k:128 * k + 128, :])
                    relch.append(t)

                VT_sb = ssb.tile([128, B * S], BF)

                # Q/K/V projections, transposed outputs [o=128, s]
                for st in range(8):
                    sl = slice(512 * st, 512 * st + 512)
                    psq = sps.tile([128, 512], F32, tag="psq")
                    psk = sps.tile([128, 512], F32, tag="psk")
                    psv = sps.tile([128, 512], F32, tag="psv")
                    for k in range(8):
                        fl = dict(start=(k == 0), stop=(k == 7))
                        nc.tensor.matmul(psq, wq_sb[:, k, :], xch[k][:, sl], **fl)
                        nc.tensor.matmul(psk, wk_sb[:, k, :], xch[k][:, sl], **fl)
                        nc.tensor.matmul(psv, wv_sb[:, k, :], xch[k][:, sl], **fl)
                    nc.vector.tensor_scalar(QT[:, sl], psq, bq_sb, SCALE, AO.add, AO.mult)
                    nc.vector.tensor_scalar_add(KT[:, sl], psk, bk_sb)
                    nc.vector.tensor_scalar_add(VT_sb[:, sl], psv, bv_sb)

                # pos projections [o=128, t=1024]
                posKT_tmp = ssb.tile([128, 2 * S], BF)
                for tt in range(2):
                    sl = slice(512 * tt, 512 * tt + 512)
                    pspk = sps.tile([128, 512], F32, tag="psq")
                    pspq = sps.tile([128, 512], F32, tag="psk")
                    for k in range(8):
                        fl = dict(start=(k == 0), stop=(k == 7))
                        nc.tensor.matmul(pspk, wk_sb[:, k, :], relch[k][:, sl], **fl)
                        nc.tensor.matmul(pspq, wq_sb[:, k, :], relch[k][:, sl], **fl)
                    nc.vector.tensor_scalar_add(posKT_tmp[:, sl], pspk, bk_sb)
                    nc.vector.tensor_scalar(posQT[:, sl], pspq, bq_sb, SCALE,
                                            AO.add, AO.mult)
                # reversed copy: posKTr[:, t] = posKT_tmp[:, 1023 - t]
                nc.vector.tensor_copy(
                    posKTr,
                    ap_of(posKT_tmp, 2 * S - 1, [[2 * S, 128], [-1, 2 * S]]))

                # V transposes -> Vaug
                for b in range(B):
                    for J in range(4):
                        pvt = sps.tile([128, 128], F32, tag="psv")
                        c0 = 512 * b + 128 * J
                        nc.tensor.matmul(pvt, VT_sb[:, c0:c0 + 128], ident,
                                         start=True, stop=True)
                        nc.vector.tensor_copy(Vaug[:, 4 * b + J, 0:64], pvt[:, 0:64])
                        nc.vector.tensor_copy(Vaug[:, 4 * b + J, 65:129], pvt[:, 64:128])

            # ---------------- attention phase ----------------
            with tc.tile_pool(name="work", bufs=1) as work, \
                 tc.tile_pool(name="dscratch", bufs=1, space="DRAM") as dscratch, \
                 tc.tile_pool(name="psa384", bufs=2, space="PSUM") as psa384, \
                 tc.tile_pool(name="psa256", bufs=2, space="PSUM") as psa256, \
                 tc.tile_pool(name="psqk", bufs=2, space="PSUM") as psqk, \
                 tc.tile_pool(name="pspv", bufs=2, space="PSUM") as pspv:

                def emit_ab(b):
                    """A/B window matmuls + fp8 shear round trip for batch b.

                    A = q . pos_k_rev windows ; B = k . pos_q windows.
                    Window per (m, I) is 640 wide, split 384+256 across two
                    PSUM banks; the two heads pack into disjoint PE row
                    groups.  gath[h][:, 0] = c2p (natural), [:, 1] = p2c^T.
                    """
                    ABsb, abflat, gath = {}, {}, {}
                    for h in range(HPC):
                        ABsb[h] = work.tile([128, 8, W], F8, name=f"ABsb{h}",
                                            tag=f"ABsb{h}", bufs=SKEW + 1)
                        abflat[h] = dscratch.tile([8 * SEG], F8,
                                                  name=f"abflat{h}",
                                                  tag=f"abflat{h}", bufs=SKEW + 1)
                        gath[h] = work.tile([128, 2, 4, 512], F8, name=f"gath{h}",
                                            tag=f"gath{h}", bufs=SKEW + 1)
                    for m in range(2):
                        lhs = QT if m == 0 else KT
                        rhs = posKTr if m == 0 else posQT
                        for I in range(4):
                            w0 = 384 - 128 * I
                            pa, pb = {}, {}
                            for h in range(HPC):
                                pa[h] = psa384.tile([128, 384], F32,
                                                    name=f"psa{h}", tag="psa")
                                pb[h] = psa256.tile([128, 256], F32,
                                                    name=f"psb{h}", tag="psb")
                            for h in range(HPC):
                                hp = slice(64 * h, 64 * h + 64)
                                lw = lhs[hp, 512 * b + 128 * I:
                                         512 * b + 128 * I + 128]
                                nc.tensor.matmul(pa[h], lw, rhs[hp, w0:w0 + 384],
                                                 start=True, stop=True,
                                                 tile_position=(64 * h, 0))
                            for h in range(HPC):
                                hp = slice(64 * h, 64 * h + 64)
                                lw = lhs[hp, 512 * b + 128 * I:
                                         512 * b + 128 * I + 128]
                                nc.tensor.matmul(pb[h], lw,
                                                 rhs[hp, w0 + 384:w0 + 640],
                                                 start=True, stop=True,
                                                 tile_position=(64 * h, 0))
                            seg = 4 * m + I
                            nc.vector.tensor_copy(ABsb[0][:, seg, 0:384], pa[0])
                            nc.vector.tensor_copy(ABsb[0][:, seg, 384:640], pb[0])
                            nc.scalar.copy(ABsb[1][:, seg, 0:384], pa[1])
                            nc.scalar.copy(ABsb[1][:, seg, 384:640], pb[1])

                    # shear round trip: one fully-contiguous write + two
                    # strided gather-reads per head.  Writes go through SWDGE
                    # (gpsimd) to keep the SP sequencer free for the reads.
                    for h in range(HPC):
                        nc.gpsimd.dma_start(
                            out=ap_of(abflat[h], 0,
                                      [[W, 128], [SEG, 8], [1, W]]),
                            in_=ABsb[h][:])
                        nc.sync.dma_start(
                            out=gath[h][:, 0],
                            in_=ap_of(abflat[h], 127,
                                      [[W - 1, 128], [SEG, 4], [1, 512]]))
                        nc.sync.dma_start(
                            out=gath[h][:, 1],
                            in_=ap_of(abflat[h], 4 * SEG + 128,
                                      [[W - 1, 128], [SEG, 4], [1, 512]]))
                    return gath

                def emit_qk(b, gath):
                    """Scores (transposed), softmax, PV for batch b."""
                    pvps = {}
                    for h in range(HPC):
                        pvps[h] = pspv.tile([65, 512], F32, name=f"pv{h}",
                                            tag=f"pv{h}", bufs=1)
                    for J in range(4):
                        qkps = {}
                        for h in range(HPC):
                            hp = slice(64 * h, 64 * h + 64)
                            qkps[h] = psqk.tile([128, 512], F32, name=f"qk{h}",
                                                tag="qk", bufs=2)
                            nc.tensor.matmul(
                                qkps[h],
                                KT[hp, 512 * b + 128 * J: 512 * b + 128 * J + 128],
                                QT[hp, 512 * b: 512 * b + 512],
                                start=True, stop=False,
                                tile_position=(64 * h, 0))
                        for h in range(HPC):
                            for I in range(4):
                                nc.tensor.matmul(
                                    qkps[h][:, 128 * I:128 * I + 128],
                                    gath[h][:, 0, I, 128 * J:128 * J + 128],
                                    ident8, start=False, stop=False,
                                    skip_group_check=True)
                            nc.tensor.matmul(qkps[h], ident8, gath[h][:, 1, J, :],
                                             start=False, stop=True)
                        for h in range(HPC):
                            PT = work.tile([128, 512], BF, name=f"PT{h}",
                                           tag=f"PT{h}", bufs=2)
                            nc.scalar.activation(
                                PT, qkps[h], AF.Exp,
                                bias=mask_sb[:, b, J:J + 1], scale=1.0)
                            nc.tensor.matmul(pvps[h],
                                             Vaug[:, 4 * b + J, 65 * h:65 * h + 65],
                                             PT, start=(J == 0), stop=(J == 3))
                    for h in range(HPC):
                        outsb = work.tile([65, 512], BF, name=f"outsb{h}",
                                          tag=f"outsb{h}", bufs=2)
                        nc.vector.tensor_copy(outsb, pvps[h])
                        nc.scalar.dma_start(out=out_d[HPC * b + h], in_=outsb)

                # software pipeline: A/B windows run SKEW batches ahead of
                # the qk/softmax phase so the shear DMA latency is hidden.
                gaths = {}
                for b in range(B + SKEW):
                    if b < B:
                        gaths[b] = emit_ab(b)
                    if b >= SKEW:
                        emit_qk(b - SKEW, gaths.pop(b - SKEW))

    nc.compile()
    return nc, names


def _get_program():
    if "prog" not in _prog_cache:
        _prog_cache["prog"] = _build_program()
    return _prog_cache["prog"]


def _host_prep(x, rel_embeddings, attn_mask, Wq, bq, Wk, bk, Wv, bv):
    import ml_dtypes
    bf = ml_dtypes.bfloat16
    x = np.asarray(x, np.float32)
    xT = np.ascontiguousarray(x.reshape(B * S, DIM).T).astype(bf)
    relT = np.ascontiguousarray(np.asarray(rel_embeddings, np.float32).T).astype(bf)
    WqT = np.asarray(Wq, np.float32).T
    WkT = np.asarray(Wk, np.float32).T
    WvT = np.asarray(Wv, np.float32).T
    mask = np.ascontiguousarray(
        np.asarray(attn_mask, np.float32).reshape(B, S))
    bq = np.asarray(bq, np.float32)
    bk = np.asarray(bk, np.float32)
    bv = np.asarray(bv, np.float32)
    maps = []
    for c in range(NCORES):
        sl = slice(128 * c, 128 * c + 128)
        maps.append({
            "xT": xT,
            "relT": relT,
            "wqT": np.ascontiguousarray(WqT[:, sl]).astype(bf),
            "wkT": np.ascontiguousarray(WkT[:, sl]).astype(bf),
            "wvT": np.ascontiguousarray(WvT[:, sl]).astype(bf),
            "bq": np.ascontiguousarray(bq[sl]),
            "bk": np.ascontiguousarray(bk[sl]),
            "bv": np.ascontiguousarray(bv[sl]),
            "mask": mask,
        })
    return maps


def kernel(x, rel_embeddings, attn_mask, Wq, bq, Wk, bk, Wv, bv):
    from concourse.bass_utils import run_bass_kernel_spmd

    nc, names = _get_program()
    maps = _host_prep(x, rel_embeddings, attn_mask, Wq, bq, Wk, bk, Wv, bv)
    in_maps = [{names[k]: v for k, v in m.items()} for m in maps]
    res = run_bass_kernel_spmd(nc, in_maps, list(range(NCORES)))
    out = np.empty((B, S, DIM), np.float32)
    for c in range(NCORES):
        o = np.asarray(res.results[c][names["out"]], np.float32)
        o = o.reshape(B, HPC, HD + 1, S)
        blk = o[:, :, 0:HD, :] / o[:, :, HD:HD + 1, :]   # [B, HPC, 64, S]
        out[:, :, 128 * c:128 * c + 128] = (
            blk.transpose(0, 3, 1, 2).reshape(B, S, 128))
    return out


# revision 7
# speedup vs baseline: 1.4085x; 1.1917x over previous
"""Self-contained Trainium2 Bass kernel: DeBERTa-style disentangled MHA.

Model (per reference):
    q = x @ Wq.T + bq ; k = x @ Wk.T + bk ; v = x @ Wv.T + bv   (per-head split)
    pos_k = rel_emb @ Wk.T + bk ; pos_q = rel_emb @ Wq.T + bq
    scores[i,j] = (q_i.k_j + A[i, i-j+s] + B[j, i-j+s]) * scale + mask
        where A[i,t] = q_i . pos_k[t],  B[j,t] = k_j . pos_q[t]
    out = softmax_j(scores) @ v

Sharding: 8-way head-parallel (2 heads/core), every core handles all 8 batch rows.
Scores are computed transposed (k index on partitions) so probs feed the PV matmul
directly; the softmax denominator comes from an appended ones-column on V.
The relative-position diagonal gathers ("shear") go through a DRAM round trip
in fp8e4m3: 640-wide windows are written with row pitch 640 and read back with
row pitch 639, which turns the per-row relative shift into a plain strided DMA.

The whole kernel is one software pipeline: the A/B window work for batches 0-1
is interleaved into the projection loop, and in steady state each (batch, J)
step issues the qk/softmax/PV work for batch b-2 interleaved with the A/B
window matmuls + PSUM drains for batch b, with the PSUM->SBUF copies split
across the Vector and Scalar engines so neither becomes the pacer and the
Scalar engine's EXPs are queued ahead of its copies.
"""

import numpy as np

B, S, DIM, H, HD = 8, 512, 1024, 16, 64
NCORES = 8
HPC = H // NCORES            # heads per core = 2
SCALE = float((HD * 3) ** -0.5)
W = 640                      # shear window width per 128-row tile
SEG = W * 128                # flat DRAM segment per (m, I) block, fp8 bytes
SKEW = 2                     # batches of A/B-window lead over the qk phase

_prog_cache = {}


def _build_program():
    import concourse.bass as bass
    import concourse.mybir as mybir
    import concourse.tile as tile
    from concourse import bacc
    from concourse.masks import make_identity

    BF = mybir.dt.bfloat16
    F8 = mybir.dt.float8e4
    F32 = mybir.dt.float32
    AO = mybir.AluOpType
    AF = mybir.ActivationFunctionType

    nc = bacc.Bacc(None, target_bir_lowering=False, debug=False)

    def ap_of(t, extra_off, dims):
        return bass.AP(t.tensor, int(t.offset) + extra_off, dims)

    names = {}

    with tile.TileContext(nc) as tc:
        with tc.tile_pool(name="dram", bufs=1, space="DRAM") as dram, \
             tc.tile_pool(name="const", bufs=1) as const, \
             tc.tile_pool(name="persist", bufs=1) as persist:

            # ---------------- I/O ----------------
            xT_d = dram.tile([DIM, B * S], BF, kind="ExternalInput", name="xT")
            relT_d = dram.tile([DIM, 2 * S], BF, kind="ExternalInput", name="relT")
            wqT_d = dram.tile([DIM, 128], BF, kind="ExternalInput", name="wqT")
            wkT_d = dram.tile([DIM, 128], BF, kind="ExternalInput", name="wkT")
            wvT_d = dram.tile([DIM, 128], BF, kind="ExternalInput", name="wvT")
            bq_d = dram.tile([128], F32, kind="ExternalInput", name="bq")
            bk_d = dram.tile([128], F32, kind="ExternalInput", name="bk")
            bv_d = dram.tile([128], F32, kind="ExternalInput", name="bv")
            mask_d = dram.tile([B, S], F32, kind="ExternalInput", name="mask")
            out_d = dram.tile([B * HPC, HD + 1, S], BF, kind="ExternalOutput",
                              name="out")
            for k, t in [("xT", xT_d), ("relT", relT_d), ("wqT", wqT_d),
                         ("wkT", wkT_d), ("wvT", wvT_d), ("bq", bq_d),
                         ("bk", bk_d), ("bv", bv_d), ("mask", mask_d),
                         ("out", out_d)]:
                names[k] = t.name

            # ---------------- persistent SBUF ----------------
            ident8 = const.tile([128, 128], F8)
            make_identity(nc, ident8)
            ident = const.tile([128, 128], BF)
            make_identity(nc, ident)
            bq_sb = const.tile([128, 1], F32)
            bk_sb = const.tile([128, 1], F32)
            bv_sb = const.tile([128, 1], F32)
            nc.sync.dma_start(out=bq_sb, in_=bq_d.rearrange("(p o) -> p o", o=1))
            nc.sync.dma_start(out=bk_sb, in_=bk_d.rearrange("(p o) -> p o", o=1))
            nc.sync.dma_start(out=bv_sb, in_=bv_d.rearrange("(p o) -> p o", o=1))
            # mask_sb[p, b*4+J] = mask[b, 128J + p]
            mask_sb = const.tile([128, B, 4], F32)
            nc.sync.dma_start(
                out=mask_sb,
                in_=ap_of(mask_d, 0, [[1, 128], [S, B], [128, 4]]))

            QT = persist.tile([128, B * S], BF)       # (x@WqT + bq)*scale, transposed
            KT = persist.tile([128, B * S], BF)       # x@WkT + bk, transposed
            posKTr = persist.tile([128, 2 * S], BF)   # pos_k^T, t-axis reversed
            posQT = persist.tile([128, 2 * S], BF)    # (pos_q^T)*scale
            # Vaug[:, b*4+J, 65h : 65h+65] = [v rows | ones] for PV lhsT
            Vaug = persist.tile([128, B * 4, 130], BF)
            nc.vector.memset(Vaug[:, :, 64:65], 1.0)
            nc.vector.memset(Vaug[:, :, 129:130], 1.0)

            from contextlib import ExitStack
            ab_pools = ExitStack()
            work = ab_pools.enter_context(tc.tile_pool(name="work", bufs=1))
            dscratch = ab_pools.enter_context(
                tc.tile_pool(name="dscratch", bufs=1, space="DRAM"))
            psa384 = ab_pools.enter_context(
                tc.tile_pool(name="psa384", bufs=2, space="PSUM"))
            psa256 = ab_pools.enter_context(
                tc.tile_pool(name="psa256", bufs=2, space="PSUM"))

            # ---- A/B window machinery (shared by prologue + steady state) ----
            abstate = {}

            def ab_begin(b):
                st = {}
                for h in range(HPC):
                    st[f"ABsb{h}"] = work.tile([128, 8, W], F8, name=f"ABsb{h}",
                                               tag=f"ABsb{h}", bufs=SKEW + 1)
                    st[f"abflat{h}"] = dscratch.tile([8 * SEG], F8,
                                                     name=f"abflat{h}",
                                                     tag=f"abflat{h}",
                                                     bufs=SKEW + 1)
                    st[f"gath{h}"] = work.tile([128, 2, 4, 512], F8,
                                               name=f"gath{h}",
                                               tag=f"gath{h}", bufs=SKEW + 1)
                abstate[b] = st

            def ab_units(b, units):
                """A/B window matmuls + drains for units u = 4m + I of batch b.

                A = q . pos_k_rev windows ; B = k . pos_q windows.
                Window per (m, I) is 640 wide, split 384+256 across two PSUM
                banks; the two heads pack into disjoint PE row groups.  The
                f32->fp8 drains alternate between DVE and ACT so neither
                engine paces the pipeline.
                """
                st = abstate[b]
                for u in units:
                    m, I = divmod(u, 4)
                    lhs = QT if m == 0 else KT
                    rhs = posKTr if m == 0 else posQT
                    w0 = 384 - 128 * I
                    pa, pb = {}, {}
                    for h in range(HPC):
                        pa[h] = psa384.tile([128, 384], F32,
                                            name=f"psa{h}", tag="psa")
                        pb[h] = psa256.tile([128, 256], F32,
                                            name=f"psb{h}", tag="psb")
                    for h in range(HPC):
                        hp = slice(64 * h, 64 * h + 64)
                        lw = lhs[hp, 512 * b + 128 * I:512 * b + 128 * I + 128]
                        nc.tensor.matmul(pa[h], lw, rhs[hp, w0:w0 + 384],
                                         start=True, stop=True,
                                         tile_position=(64 * h, 0))
                    for h in range(HPC):
                        hp = slice(64 * h, 64 * h + 64)
                        lw = lhs[hp, 512 * b + 128 * I:512 * b + 128 * I + 128]
                        nc.tensor.matmul(pb[h], lw, rhs[hp, w0 + 384:w0 + 640],
                                         start=True, stop=True,
                                         tile_position=(64 * h, 0))
                    ABsb0, ABsb1 = st["ABsb0"], st["ABsb1"]
                    nc.vector.tensor_copy(ABsb0[:, u, 0:384], pa[0])
                    nc.vector.tensor_copy(ABsb0[:, u, 384:640], pb[0])
                    nc.scalar.copy(ABsb1[:, u, 0:384], pa[1])
                    if u % 2 == 0:
                        nc.scalar.copy(ABsb1[:, u, 384:640], pb[1])
                    else:
                        nc.vector.tensor_copy(ABsb1[:, u, 384:640], pb[1])

            def ab_write(b, m):
                """Shear write of half m for batch b + the matching gather read.

                Writes go through SWDGE (gpsimd) to keep the SP sequencer free
                for the strided reads.  gath[h][:, 0] = c2p (natural),
                gath[h][:, 1] = p2c^T.
                """
                st = abstate[b]
                for h in range(HPC):
                    nc.gpsimd.dma_start(
                        out=ap_of(st[f"abflat{h}"], 4 * SEG * m,
                                  [[W, 128], [SEG, 4], [1, W]]),
                        in_=st[f"ABsb{h}"][:, 4 * m:4 * m + 4])
                    nc.sync.dma_start(
                        out=st[f"gath{h}"][:, m],
                        in_=ap_of(st[f"abflat{h}"], 4 * SEG * m + 127 + m,
                                  [[W - 1, 128], [SEG, 4], [1, 512]]))

            # ---------------- setup phase ----------------
            with tc.tile_pool(name="wpool", bufs=1) as wpool, \
                 tc.tile_pool(name="xpool", bufs=1) as xpool, \
                 tc.tile_pool(name="setup_sb", bufs=1) as ssb, \
                 tc.tile_pool(name="setup_ps", bufs=1, space="PSUM") as sps:

                wq_sb = wpool.tile([128, 8, 128], BF)
                wk_sb = wpool.tile([128, 8, 128], BF)
                wv_sb = wpool.tile([128, 8, 128], BF)
                for wsb, wd in [(wq_sb, wqT_d), (wk_sb, wkT_d), (wv_sb, wvT_d)]:
                    nc.sync.dma_start(
                        out=wsb, in_=wd.rearrange("(k p) o -> p k o", p=128))
                relch = []
                for k in range(8):
                    t = ssb.tile([128, 2 * S], BF, name=f"relch{k}", tag=f"relch{k}")
                    nc.sync.dma_start(out=t, in_=relT_d[128 * k:128 * k + 128, :])
                    relch.append(t)
                xch = []
                for k in range(8):
                    t = xpool.tile([128, B * S], BF, name=f"xch{k}", tag=f"xch{k}")
                    nc.sync.dma_start(out=t, in_=xT_d[128 * k:128 * k + 128, :])
                    xch.append(t)

                VT_sb = ssb.tile([128, B * S], BF)

                # pos projections first: [o=128, t=1024]
                posKT_tmp = ssb.tile([128, 2 * S], BF)
                for tt in range(2):
                    sl = slice(512 * tt, 512 * tt + 512)
                    pspk = sps.tile([128, 512], F32, tag="psq")
                    pspq = sps.tile([128, 512], F32, tag="psk")
                    for k in range(8):
                        fl = dict(start=(k == 0), stop=(k == 7))
                        nc.tensor.matmul(pspk, wk_sb[:, k, :], relch[k][:, sl], **fl)
                        nc.tensor.matmul(pspq, wq_sb[:, k, :], relch[k][:, sl], **fl)
                    nc.vector.tensor_scalar_add(posKT_tmp[:, sl], pspk, bk_sb)
                    nc.vector.tensor_scalar(posQT[:, sl], pspq, bq_sb, SCALE,
                                            AO.add, AO.mult)
                # reversed copy: posKTr[:, t] = posKT_tmp[:, 1023 - t]
                nc.vector.tensor_copy(
                    posKTr,
                    ap_of(posKT_tmp, 2 * S - 1, [[2 * S, 128], [-1, 2 * S]]))

                def proj(st):
                    sl = slice(512 * st, 512 * st + 512)
                    psq = sps.tile([128, 512], F32, tag="psq")
                    psk = sps.tile([128, 512], F32, tag="psk")
                    psv = sps.tile([128, 512], F32, tag="psv")
                    for k in range(8):
                        fl = dict(start=(k == 0), stop=(k == 7))
                        nc.tensor.matmul(psq, wq_sb[:, k, :], xch[k][:, sl], **fl)
                        nc.tensor.matmul(psk, wk_sb[:, k, :], xch[k][:, sl], **fl)
                        nc.tensor.matmul(psv, wv_sb[:, k, :], xch[k][:, sl], **fl)
                    nc.vector.tensor_scalar(QT[:, sl], psq, bq_sb, SCALE,
                                            AO.add, AO.mult)
                    nc.vector.tensor_scalar_add(KT[:, sl], psk, bk_sb)
                    nc.vector.tensor_scalar_add(VT_sb[:, sl], psv, bv_sb)

                # Q/K/V projections with the A/B window work for batches 0-1
                # interleaved so the drain engines and shear DMA start early.
                proj(0)
                ab_begin(0)
                proj(1)
                ab_units(0, [0, 1])
                proj(2)
                ab_units(0, [2, 3])
                ab_write(0, 0)
                proj(3)
                ab_units(0, [4, 5])
                proj(4)
                ab_units(0, [6, 7])
                ab_write(0, 1)
                ab_begin(1)
                proj(5)
                ab_units(1, [0, 1])
                proj(6)
                ab_units(1, [2, 3])
                ab_write(1, 0)
                proj(7)
                ab_units(1, [4, 5])
                ab_units(1, [6, 7])
                ab_write(1, 1)

                # V transposes -> Vaug; 4 J-tiles share one PSUM bank so the
                # drain is a single strided copy per batch row.
                for b in range(B):
                    pvt4 = sps.tile([128, 512], F32, tag="psv")
                    for J in range(4):
                        c0 = 512 * b + 128 * J
                        nc.tensor.matmul(pvt4[:, 128 * J:128 * J + 128],
                                         VT_sb[:, c0:c0 + 128], ident,
                                         start=(J == 0), stop=(J == 3),
                                         skip_group_check=True)
                    nc.vector.tensor_copy(
                        ap_of(Vaug, (4 * b) * 130,
                              [[B * 4 * 130, 128], [130, 4], [65, 2], [1, 64]]),
                        pvt4.rearrange("p (j h d) -> p j h d", j=4, h=2))

            # ---------------- attention phase ----------------
            with tc.tile_pool(name="psqk", bufs=2, space="PSUM") as psqk, \
                 tc.tile_pool(name="pspv", bufs=2, space="PSUM") as pspv:

                pvtiles = {}

                def qk_step(b, J):
                    """Scores (transposed) for (b, J), softmax, PV."""
                    st = abstate[b]
                    if J == 0:
                        pvtiles[b] = {
                            h: pspv.tile([65, 512], F32, name=f"pv{h}",
                                         tag=f"pv{h}", bufs=1)
                            for h in range(HPC)}
                    pvps = pvtiles[b]
                    qkps, PT = {}, {}
                    for h in range(HPC):
                        hp = slice(64 * h, 64 * h + 64)
                        qkps[h] = psqk.tile([128, 512], F32, name=f"qk{h}",
                                            tag="qk", bufs=2)
                        nc.tensor.matmul(
                            qkps[h],
                            KT[hp, 512 * b + 128 * J: 512 * b + 128 * J + 128],
                            QT[hp, 512 * b: 512 * b + 512],
                            start=True, stop=False,
                            tile_position=(64 * h, 0))
                    for h in range(HPC):
                        gath = st[f"gath{h}"]
                        for I in range(4):
                            nc.tensor.matmul(
                                qkps[h][:, 128 * I:128 * I + 128],
                                gath[:, 0, I, 128 * J:128 * J + 128],
                                ident8, start=False, stop=False,
                                skip_group_check=True)
                        nc.tensor.matmul(qkps[h], ident8, gath[:, 1, J, :],
                                         start=False, stop=True)
                        PT[h] = work.tile([128, 512], BF, name=f"PT{h}",
                                          tag=f"PT{h}", bufs=2)
                        nc.scalar.activation(
                            PT[h], qkps[h], AF.Exp,
                            bias=mask_sb[:, b, J:J + 1], scale=1.0)
                    return PT, pvps

                def pv_step(b, J, PT, pvps):
                    for h in range(HPC):
                        nc.tensor.matmul(pvps[h],
                                         Vaug[:, 4 * b + J, 65 * h:65 * h + 65],
                                         PT[h], start=(J == 0), stop=(J == 3))

                def out_step(b, pvps):
                    for h in range(HPC):
                        outsb = work.tile([65, 512], BF, name=f"outsb{h}",
                                          tag=f"outsb{h}", bufs=2)
                        nc.vector.tensor_copy(outsb, pvps[h])
                        nc.scalar.dma_start(out=out_d[HPC * b + h], in_=outsb)
                    del pvtiles[b]

                # steady state: qk phase for batch b-SKEW interleaved with the
                # A/B window phase for batch b, one J-step at a time.
                for b in range(SKEW, B + SKEW):
                    if b < B:
                        ab_begin(b)
                    for J in range(4):
                        PT, pvps = qk_step(b - SKEW, J)
                        if b < B:
                            ab_units(b, [2 * J, 2 * J + 1])
                            if J == 1:
                                ab_write(b, 0)
                        pv_step(b - SKEW, J, PT, pvps)
                    if b < B:
                        ab_write(b, 1)
                    out_step(b - SKEW, pvps)
                    del abstate[b - SKEW]

            ab_pools.close()

    nc.compile()
    return nc, names


def _get_program():
    if "prog" not in _prog_cache:
        _prog_cache["prog"] = _build_program()
    return _prog_cache["prog"]


def _host_prep(x, rel_embeddings, attn_mask, Wq, bq, Wk, bk, Wv, bv):
    import ml_dtypes
    bf = ml_dtypes.bfloat16
    x = np.asarray(x, np.float32)
    xT = np.ascontiguousarray(x.reshape(B * S, DIM).T).astype(bf)
    relT = np.ascontiguousarray(np.asarray(rel_embeddings, np.float32).T).astype(bf)
    WqT = np.asarray(Wq, np.float32).T
    WkT = np.asarray(Wk, np.float32).T
    WvT = np.asarray(Wv, np.float32).T
    mask = np.ascontiguousarray(
        np.asarray(attn_mask, np.float32).reshape(B, S))
    bq = np.asarray(bq, np.float32)
    bk = np.asarray(bk, np.float32)
    bv = np.asarray(bv, np.float32)
    maps = []
    for c in range(NCORES):
        sl = slice(128 * c, 128 * c + 128)
        maps.append({
            "xT": xT,
            "relT": relT,
            "wqT": np.ascontiguousarray(WqT[:, sl]).astype(bf),
            "wkT": np.ascontiguousarray(WkT[:, sl]).astype(bf),
            "wvT": np.ascontiguousarray(WvT[:, sl]).astype(bf),
            "bq": np.ascontiguousarray(bq[sl]),
            "bk": np.ascontiguousarray(bk[sl]),
            "bv": np.ascontiguousarray(bv[sl]),
            "mask": mask,
        })
    return maps


def kernel(x, rel_embeddings, attn_mask, Wq, bq, Wk, bk, Wv, bv):
    from concourse.bass_utils import run_bass_kernel_spmd

    nc, names = _get_program()
    maps = _host_prep(x, rel_embeddings, attn_mask, Wq, bq, Wk, bk, Wv, bv)
    in_maps = [{names[k]: v for k, v in m.items()} for m in maps]
    res = run_bass_kernel_spmd(nc, in_maps, list(range(NCORES)))
    out = np.empty((B, S, DIM), np.float32)
    for c in range(NCORES):
        o = np.asarray(res.results[c][names["out"]], np.float32)
        o = o.reshape(B, HPC, HD + 1, S)
        blk = o[:, :, 0:HD, :] / o[:, :, HD:HD + 1, :]   # [B, HPC, 64, S]
        out[:, :, 128 * c:128 * c + 128] = (
            blk.transpose(0, 3, 1, 2).reshape(B, S, 128))
    return out


# revision 8
# speedup vs baseline: 1.5151x; 1.0756x over previous
"""Self-contained Trainium2 Bass kernel: DeBERTa-style disentangled MHA.

Model (per reference):
    q = x @ Wq.T + bq ; k = x @ Wk.T + bk ; v = x @ Wv.T + bv   (per-head split)
    pos_k = rel_emb @ Wk.T + bk ; pos_q = rel_emb @ Wq.T + bq
    scores[i,j] = (q_i.k_j + A[i, i-j+s] + B[j, i-j+s]) * scale + mask
        where A[i,t] = q_i . pos_k[t],  B[j,t] = k_j . pos_q[t]
    out = softmax_j(scores) @ v

Sharding: 8-way head-parallel (2 heads/core), every core handles all 8 batch rows.
Scores are computed transposed (k index on partitions) so probs feed the PV matmul
directly; the softmax denominator comes from an appended ones-column on V.
The relative-position diagonal gathers ("shear") go through a DRAM round trip
in fp8e4m3: 640-wide windows are written with row pitch 640 and read back with
row pitch 639, which turns the per-row relative shift into a plain strided DMA.

The whole kernel is one software pipeline: the A/B window work for batches 0-1
is interleaved into the projection loop, and in steady state each (batch, J)
step issues the qk/softmax/PV work for batch b-2 interleaved with the A/B
window matmuls + PSUM drains for batch b, with the PSUM->SBUF copies split
across the Vector and Scalar engines so neither becomes the pacer and the
Scalar engine's EXPs are queued ahead of its copies.
"""

import numpy as np

B, S, DIM, H, HD = 8, 512, 1024, 16, 64
NCORES = 8
HPC = H // NCORES            # heads per core = 2
SCALE = float((HD * 3) ** -0.5)
W = 640                      # shear window width per 128-row tile
SEG = W * 128                # flat DRAM segment per (m, I) block, fp8 bytes
SKEW = 2                     # batches of A/B-window lead over the qk phase

_prog_cache = {}


def _build_program():
    import concourse.bass as bass
    import concourse.mybir as mybir
    import concourse.tile as tile
    from concourse import bacc
    from concourse.masks import make_identity

    BF = mybir.dt.bfloat16
    F8 = mybir.dt.float8e4
    F32 = mybir.dt.float32
    AO = mybir.AluOpType
    AF = mybir.ActivationFunctionType

    nc = bacc.Bacc(None, target_bir_lowering=False, debug=False)

    def ap_of(t, extra_off, dims):
        return bass.AP(t.tensor, int(t.offset) + extra_off, dims)

    names = {}

    with tile.TileContext(nc) as tc:
        with tc.tile_pool(name="dram", bufs=1, space="DRAM") as dram, \
             tc.tile_pool(name="const", bufs=1) as const, \
             tc.tile_pool(name="persist", bufs=1) as persist:

            # ---------------- I/O ----------------
            xT_d = dram.tile([DIM, B * S], BF, kind="ExternalInput", name="xT")
            relT_d = dram.tile([DIM, 2 * S], BF, kind="ExternalInput", name="relT")
            wqT_d = dram.tile([DIM, 128], BF, kind="ExternalInput", name="wqT")
            wkT_d = dram.tile([DIM, 128], BF, kind="ExternalInput", name="wkT")
            wvT_d = dram.tile([DIM, 128], BF, kind="ExternalInput", name="wvT")
            bq_d = dram.tile([128], F32, kind="ExternalInput", name="bq")
            bk_d = dram.tile([128], F32, kind="ExternalInput", name="bk")
            bv_d = dram.tile([128], F32, kind="ExternalInput", name="bv")
            mask_d = dram.tile([B, S], F32, kind="ExternalInput", name="mask")
            out_d = dram.tile([B * HPC, HD + 1, S], BF, kind="ExternalOutput",
                              name="out")
            for k, t in [("xT", xT_d), ("relT", relT_d), ("wqT", wqT_d),
                         ("wkT", wkT_d), ("wvT", wvT_d), ("bq", bq_d),
                         ("bk", bk_d), ("bv", bv_d), ("mask", mask_d),
                         ("out", out_d)]:
                names[k] = t.name

            # ---------------- persistent SBUF ----------------
            ident8 = const.tile([128, 128], F8)
            make_identity(nc, ident8)
            ident = const.tile([128, 128], BF)
            make_identity(nc, ident)
            bq_sb = const.tile([128, 1], F32)
            bk_sb = const.tile([128, 1], F32)
            bv_sb = const.tile([128, 1], F32)
            nc.sync.dma_start(out=bq_sb, in_=bq_d.rearrange("(p o) -> p o", o=1))
            nc.sync.dma_start(out=bk_sb, in_=bk_d.rearrange("(p o) -> p o", o=1))
            nc.sync.dma_start(out=bv_sb, in_=bv_d.rearrange("(p o) -> p o", o=1))
            # mask_sb[p, b*4+J] = mask[b, 128J + p]
            mask_sb = const.tile([128, B, 4], F32)
            nc.sync.dma_start(
                out=mask_sb,
                in_=ap_of(mask_d, 0, [[1, 128], [S, B], [128, 4]]))

            QT = persist.tile([128, B * S], BF)       # (x@WqT + bq)*scale, transposed
            KT = persist.tile([128, B * S], BF)       # x@WkT + bk, transposed
            posKTr = persist.tile([128, 2 * S], BF)   # pos_k^T, t-axis reversed
            posQT = persist.tile([128, 2 * S], BF)    # (pos_q^T)*scale
            # Vaug[:, b*4+J, 65h : 65h+65] = [v rows | ones] for PV lhsT
            Vaug = persist.tile([128, B * 4, 130], BF)
            nc.vector.memset(Vaug[:, :, 64:65], 1.0)
            nc.vector.memset(Vaug[:, :, 129:130], 1.0)

            from contextlib import ExitStack
            ab_pools = ExitStack()
            work = ab_pools.enter_context(tc.tile_pool(name="work", bufs=1))
            dscratch = ab_pools.enter_context(
                tc.tile_pool(name="dscratch", bufs=1, space="DRAM"))
            psa384 = ab_pools.enter_context(
                tc.tile_pool(name="psa384", bufs=2, space="PSUM"))
            psa256 = ab_pools.enter_context(
                tc.tile_pool(name="psa256", bufs=2, space="PSUM"))

            # ---- A/B window machinery (shared by prologue + steady state) ----
            abstate = {}

            def ab_begin(b):
                st = {}
                for h in range(HPC):
                    st[f"ABsb{h}"] = work.tile([128, 8, W], F8, name=f"ABsb{h}",
                                               tag=f"ABsb{h}", bufs=SKEW + 1)
                    st[f"abflat{h}"] = dscratch.tile([8 * SEG], F8,
                                                     name=f"abflat{h}",
                                                     tag=f"abflat{h}",
                                                     bufs=SKEW + 1)
                    st[f"gath{h}"] = work.tile([128, 2, 4, 512], F8,
                                               name=f"gath{h}",
                                               tag=f"gath{h}", bufs=SKEW + 1)
                abstate[b] = st

            def ab_units(b, units):
                """A/B window matmuls + drains for units u = 4m + I of batch b.

                A = q . pos_k_rev windows ; B = k . pos_q windows.
                Window per (m, I) is 640 wide, split 384+256 across two PSUM
                banks; the two heads pack into disjoint PE row groups.  The
                f32->fp8 drains alternate between DVE and ACT so neither
                engine paces the pipeline.
                """
                st = abstate[b]
                for u in units:
                    m, I = divmod(u, 4)
                    lhs = QT if m == 0 else KT
                    rhs = posKTr if m == 0 else posQT
                    w0 = 384 - 128 * I
                    pa, pb = {}, {}
                    for h in range(HPC):
                        pa[h] = psa384.tile([128, 384], F32,
                                            name=f"psa{h}", tag="psa")
                        pb[h] = psa256.tile([128, 256], F32,
                                            name=f"psb{h}", tag="psb")
                    for h in range(HPC):
                        hp = slice(64 * h, 64 * h + 64)
                        lw = lhs[hp, 512 * b + 128 * I:512 * b + 128 * I + 128]
                        nc.tensor.matmul(pa[h], lw, rhs[hp, w0:w0 + 384],
                                         start=True, stop=True,
                                         tile_position=(64 * h, 0))
                    for h in range(HPC):
                        hp = slice(64 * h, 64 * h + 64)
                        lw = lhs[hp, 512 * b + 128 * I:512 * b + 128 * I + 128]
                        nc.tensor.matmul(pb[h], lw, rhs[hp, w0 + 384:w0 + 640],
                                         start=True, stop=True,
                                         tile_position=(64 * h, 0))
                    ABsb0, ABsb1 = st["ABsb0"], st["ABsb1"]
                    nc.vector.tensor_copy(ABsb0[:, u, 0:384], pa[0])
                    nc.vector.tensor_copy(ABsb0[:, u, 384:640], pb[0])
                    nc.scalar.copy(ABsb1[:, u, 0:384], pa[1])
                    if u % 2 == 0:
                        nc.scalar.copy(ABsb1[:, u, 384:640], pb[1])
                    else:
                        nc.vector.tensor_copy(ABsb1[:, u, 384:640], pb[1])

            def ab_write(b, m):
                """Shear write of half m for batch b + the matching gather read.

                Writes go through SWDGE (gpsimd) to keep the SP sequencer free
                for the strided reads.  gath[h][:, 0] = c2p (natural),
                gath[h][:, 1] = p2c^T.
                """
                st = abstate[b]
                for h in range(HPC):
                    nc.gpsimd.dma_start(
                        out=ap_of(st[f"abflat{h}"], 4 * SEG * m,
                                  [[W, 128], [SEG, 4], [1, W]]),
                        in_=st[f"ABsb{h}"][:, 4 * m:4 * m + 4])
                    nc.sync.dma_start(
                        out=st[f"gath{h}"][:, m],
                        in_=ap_of(st[f"abflat{h}"], 4 * SEG * m + 127 + m,
                                  [[W - 1, 128], [SEG, 4], [1, 512]]))

            # ---------------- setup phase ----------------
            with tc.tile_pool(name="wpool", bufs=1) as wpool, \
                 tc.tile_pool(name="xpool", bufs=1) as xpool, \
                 tc.tile_pool(name="setup_sb", bufs=1) as ssb, \
                 tc.tile_pool(name="setup_ps", bufs=1, space="PSUM") as sps:

                wq_sb = wpool.tile([128, 8, 128], BF)
                wk_sb = wpool.tile([128, 8, 128], BF)
                wv_sb = wpool.tile([128, 8, 128], BF)
                for wsb, wd in [(wq_sb, wqT_d), (wk_sb, wkT_d), (wv_sb, wvT_d)]:
                    nc.sync.dma_start(
                        out=wsb, in_=wd.rearrange("(k p) o -> p k o", p=128))
                relch = []
                for k in range(8):
                    t = ssb.tile([128, 2 * S], BF, name=f"relch{k}", tag=f"relch{k}")
                    nc.sync.dma_start(out=t, in_=relT_d[128 * k:128 * k + 128, :])
                    relch.append(t)
                # x loads go token-segment-major so proj(0) can start after
                # ~1MB instead of waiting for the whole 8MB transfer.
                xch = [xpool.tile([128, B * S], BF, name=f"xch{k}", tag=f"xch{k}")
                       for k in range(8)]
                for st in range(8):
                    sl = slice(512 * st, 512 * st + 512)
                    for k in range(8):
                        nc.sync.dma_start(out=xch[k][:, sl],
                                          in_=xT_d[128 * k:128 * k + 128, sl])

                VT_sb = ssb.tile([128, B * S], BF)

                # pos projections first: [o=128, t=1024]
                posKT_tmp = ssb.tile([128, 2 * S], BF)
                for tt in range(2):
                    sl = slice(512 * tt, 512 * tt + 512)
                    pspk = sps.tile([128, 512], F32, tag="psq")
                    pspq = sps.tile([128, 512], F32, tag="psk")
                    for k in range(8):
                        fl = dict(start=(k == 0), stop=(k == 7))
                        nc.tensor.matmul(pspk, wk_sb[:, k, :], relch[k][:, sl], **fl)
                        nc.tensor.matmul(pspq, wq_sb[:, k, :], relch[k][:, sl], **fl)
                    nc.vector.tensor_scalar_add(posKT_tmp[:, sl], pspk, bk_sb)
                    nc.vector.tensor_scalar(posQT[:, sl], pspq, bq_sb, SCALE,
                                            AO.add, AO.mult)
                # reversed copy: posKTr[:, t] = posKT_tmp[:, 1023 - t]
                nc.vector.tensor_copy(
                    posKTr,
                    ap_of(posKT_tmp, 2 * S - 1, [[2 * S, 128], [-1, 2 * S]]))

                def proj(st):
                    sl = slice(512 * st, 512 * st + 512)
                    psq = sps.tile([128, 512], F32, tag="psq")
                    psk = sps.tile([128, 512], F32, tag="psk")
                    psv = sps.tile([128, 512], F32, tag="psv")
                    for k in range(8):
                        fl = dict(start=(k == 0), stop=(k == 7))
                        nc.tensor.matmul(psq, wq_sb[:, k, :], xch[k][:, sl], **fl)
                        nc.tensor.matmul(psk, wk_sb[:, k, :], xch[k][:, sl], **fl)
                        nc.tensor.matmul(psv, wv_sb[:, k, :], xch[k][:, sl], **fl)
                    nc.vector.tensor_scalar(QT[:, sl], psq, bq_sb, SCALE,
                                            AO.add, AO.mult)
                    nc.vector.tensor_scalar_add(KT[:, sl], psk, bk_sb)
                    nc.vector.tensor_scalar_add(VT_sb[:, sl], psv, bv_sb)

                # Q/K/V projections with the A/B window work for batches 0-1
                # interleaved so the drain engines and shear DMA start early.
                proj(0)
                ab_begin(0)
                proj(1)
                ab_units(0, [0, 1])
                proj(2)
                ab_units(0, [2, 3])
                ab_write(0, 0)
                proj(3)
                ab_units(0, [4, 5])
                proj(4)
                ab_units(0, [6, 7])
                ab_write(0, 1)
                ab_begin(1)
                proj(5)
                ab_units(1, [0, 1])
                proj(6)
                ab_units(1, [2, 3])
                ab_write(1, 0)
                proj(7)
                ab_units(1, [4, 5])
                ab_units(1, [6, 7])
                ab_write(1, 1)

                # V transposes -> Vaug; 4 J-tiles share one PSUM bank so the
                # drain is a single strided copy per batch row.
                for b in range(B):
                    pvt4 = sps.tile([128, 512], F32, tag="psv")
                    for J in range(4):
                        c0 = 512 * b + 128 * J
                        nc.tensor.matmul(pvt4[:, 128 * J:128 * J + 128],
                                         VT_sb[:, c0:c0 + 128], ident,
                                         start=(J == 0), stop=(J == 3),
                                         skip_group_check=True)
                    nc.vector.tensor_copy(
                        ap_of(Vaug, (4 * b) * 130,
                              [[B * 4 * 130, 128], [130, 4], [65, 2], [1, 64]]),
                        pvt4.rearrange("p (j h d) -> p j h d", j=4, h=2))

            # ---------------- attention phase ----------------
            with tc.tile_pool(name="psqk", bufs=2, space="PSUM") as psqk, \
                 tc.tile_pool(name="pspv", bufs=2, space="PSUM") as pspv:

                pvtiles = {}

                def qk_step(b, J):
                    """Scores (transposed) for (b, J), softmax, PV."""
                    st = abstate[b]
                    if J == 0:
                        pvtiles[b] = {
                            h: pspv.tile([65, 512], F32, name=f"pv{h}",
                                         tag=f"pv{h}", bufs=1)
                            for h in range(HPC)}
                    pvps = pvtiles[b]
                    qkps, PT = {}, {}
                    for h in range(HPC):
                        hp = slice(64 * h, 64 * h + 64)
                        qkps[h] = psqk.tile([128, 512], F32, name=f"qk{h}",
                                            tag="qk", bufs=2)
                        nc.tensor.matmul(
                            qkps[h],
                            KT[hp, 512 * b + 128 * J: 512 * b + 128 * J + 128],
                            QT[hp, 512 * b: 512 * b + 512],
                            start=True, stop=False,
                            tile_position=(64 * h, 0))
                    for h in range(HPC):
                        gath = st[f"gath{h}"]
                        for I in range(4):
                            nc.tensor.matmul(
                                qkps[h][:, 128 * I:128 * I + 128],
                                gath[:, 0, I, 128 * J:128 * J + 128],
                                ident8, start=False, stop=False,
                                skip_group_check=True)
                        nc.tensor.matmul(qkps[h], ident8, gath[:, 1, J, :],
                                         start=False, stop=True)
                        PT[h] = work.tile([128, 512], BF, name=f"PT{h}",
                                          tag=f"PT{h}", bufs=2)
                        nc.scalar.activation(
                            PT[h], qkps[h], AF.Exp,
                            bias=mask_sb[:, b, J:J + 1], scale=1.0)
                    return PT, pvps

                def pv_step(b, J, PT, pvps):
                    for h in range(HPC):
                        nc.tensor.matmul(pvps[h],
                                         Vaug[:, 4 * b + J, 65 * h:65 * h + 65],
                                         PT[h], start=(J == 0), stop=(J == 3))

                def out_step(b, pvps):
                    for h in range(HPC):
                        outsb = work.tile([65, 512], BF, name=f"outsb{h}",
                                          tag=f"outsb{h}", bufs=2)
                        nc.vector.tensor_copy(outsb, pvps[h])
                        nc.scalar.dma_start(out=out_d[HPC * b + h], in_=outsb)
                    del pvtiles[b]

                # steady state: qk phase for batch b-SKEW interleaved with the
                # A/B window phase for batch b, one J-step at a time.
                for b in range(SKEW, B + SKEW):
                    if b < B:
                        ab_begin(b)
                    for J in range(4):
                        PT, pvps = qk_step(b - SKEW, J)
                        if b < B:
                            ab_units(b, [2 * J, 2 * J + 1])
                            if J == 1:
                                ab_write(b, 0)
                        pv_step(b - SKEW, J, PT, pvps)
                    if b < B:
                        ab_write(b, 1)
                    out_step(b - SKEW, pvps)
                    del abstate[b - SKEW]

            ab_pools.close()

    nc.compile()
    return nc, names


def _get_program():
    if "prog" not in _prog_cache:
        _prog_cache["prog"] = _build_program()
    return _prog_cache["prog"]


def _host_prep(x, rel_embeddings, attn_mask, Wq, bq, Wk, bk, Wv, bv):
    import ml_dtypes
    bf = ml_dtypes.bfloat16
    x = np.asarray(x, np.float32)
    xT = np.ascontiguousarray(x.reshape(B * S, DIM).T).astype(bf)
    relT = np.ascontiguousarray(np.asarray(rel_embeddings, np.float32).T).astype(bf)
    WqT = np.asarray(Wq, np.float32).T
    WkT = np.asarray(Wk, np.float32).T
    WvT = np.asarray(Wv, np.float32).T
    mask = np.ascontiguousarray(
        np.asarray(attn_mask, np.float32).reshape(B, S))
    bq = np.asarray(bq, np.float32)
    bk = np.asarray(bk, np.float32)
    bv = np.asarray(bv, np.float32)
    maps = []
    for c in range(NCORES):
        sl = slice(128 * c, 128 * c + 128)
        maps.append({
            "xT": xT,
            "relT": relT,
            "wqT": np.ascontiguousarray(WqT[:, sl]).astype(bf),
            "wkT": np.ascontiguousarray(WkT[:, sl]).astype(bf),
            "wvT": np.ascontiguousarray(WvT[:, sl]).astype(bf),
            "bq": np.ascontiguousarray(bq[sl]),
            "bk": np.ascontiguousarray(bk[sl]),
            "bv": np.ascontiguousarray(bv[sl]),
            "mask": mask,
        })
    return maps


def kernel(x, rel_embeddings, attn_mask, Wq, bq, Wk, bk, Wv, bv):
    from concourse.bass_utils import run_bass_kernel_spmd

    nc, names = _get_program()
    maps = _host_prep(x, rel_embeddings, attn_mask, Wq, bq, Wk, bk, Wv, bv)
    in_maps = [{names[k]: v for k, v in m.items()} for m in maps]
    res = run_bass_kernel_spmd(nc, in_maps, list(range(NCORES)))
    out = np.empty((B, S, DIM), np.float32)
    for c in range(NCORES):
        o = np.asarray(res.results[c][names["out"]], np.float32)
        o = o.reshape(B, HPC, HD + 1, S)
        blk = o[:, :, 0:HD, :] / o[:, :, HD:HD + 1, :]   # [B, HPC, 64, S]
        out[:, :, 128 * c:128 * c + 128] = (
            blk.transpose(0, 3, 1, 2).reshape(B, S, 128))
    return out
